# revision 21
# baseline (speedup 1.0000x reference)
"""Trainium2 Bass kernel for the chunked-scan final-state problem.

Math: the reference's chunked scan + inter-chunk segsum reduces exactly to
    out[b, h, p, n] = sum_t exp(sum_{t'>t} A[b, t', h]) * X[b, t, h, p] * B[b, t, h, n]
(input C is unused by the reference).  Per (b, h) this is a (64 x 2048) @
(2048 x 16) matmul with a decay weight folded into B.

Distribution: data-parallel over batch, 8 batches per core, 8 cores.

Layout trick ("comb" K-tiles): contraction tile i takes t in {16q + i},
q = partition.  Then every DMA is fully contiguous (partition q reads rows
16q..16q+15: X 8KB/quarter, B 8KB; A is host-combed so one 512KB DMA with
4KB runs loads every batch) and the decay suffix-sum becomes
  w[q, (i,h)] = exp( suffix_i(A_row q) + carry[q, h] )
where suffix_i is a 4-step shifted-add scan along the free dim and
carry = Lstrict^T @ row_totals is one small PE matmul over partitions.

Main matmuls (per batch, 16 K-tiles): stationary = weighted-B tile
(128 x 128 = all 8 heads), moving = X tile (128 x 512) -> PSUM (128 x 512)
accumulated over i; entry ((h'n), (h''p)).  Matmuls run in float32r
(single-pass fp32, 4x the plain-fp32 rate).  The diagonal h'=h'' blocks
are the per-head outputs in (n, p) orientation; four DVE 32x32 blockwise
band transposes gather them into one 32-partition stage tile, written to
DRAM with a single contiguous DMA per batch (host decodes the layout).
"""

import numpy as np

import concourse.bacc as bacc
import concourse.mybir as mybir
import concourse.tile as tile
from concourse.bass_utils import run_bass_kernel_spmd
from concourse.masks import make_lower_triangular

F32 = mybir.dt.float32
F32R = mybir.dt.float32r
NCORES = 8
NB = 8          # batches per core
T = 2048        # sequence length
NH = 8          # heads
DP = 64         # d_head
DN = 16         # d_state
NT = T // 128   # K-tiles of 128

_NC_CACHE = None


def _build():
    global _NC_CACHE
    if _NC_CACHE is not None:
        return _NC_CACHE

    nc = bacc.Bacc("TRN2", target_bir_lowering=False, debug=False)
    Xd = nc.dram_tensor("X", (NB, T, NH, DP), F32, kind="ExternalInput").ap()
    # A is pre-combed on the host: A_dev[q, b, i, h] = A[b, 16 q + i, h]
    # -> one DMA, 4 KB contiguous per partition
    Ad = nc.dram_tensor("A", (128, NB, NT, NH), F32, kind="ExternalInput").ap()
    Bd = nc.dram_tensor("B", (NB, T, NH, DN), F32, kind="ExternalInput").ap()
    # raw staged output: O[b, pp, c] = stage band dump; host decodes
    #   out[b, 2j+e, 32k+pp, n] = O[b, pp, 128j + 64e + 32k + 16e + n]
    Od = nc.dram_tensor("O", (NB, 32, 512), F32, kind="ExternalOutput").ap()

    with tile.TileContext(nc) as tc:
        with (
            tc.tile_pool(name="consts", bufs=1) as cpool,
            tc.tile_pool(name="scan", bufs=1) as spool,
            tc.tile_pool(name="wexp", bufs=3) as wpool,
            tc.tile_pool(name="bmat", bufs=3) as bpool,
            tc.tile_pool(name="bwp", bufs=3) as bwpool,
            tc.tile_pool(name="xmat", bufs=3) as xpool,
            tc.tile_pool(name="outs", bufs=3) as opool,
            tc.tile_pool(name="ps_carry", bufs=2, space="PSUM") as pcpool,
            tc.tile_pool(name="ps_main", bufs=4, space="PSUM") as pmpool,
        ):
            # strict lower-triangular constant: L[k, m] = 1 iff k > m
            ltri = cpool.tile([128, 128], F32)
            make_lower_triangular(nc, ltri[:], val=1.0, diag=False)

            # ---- A load: all batches in one contiguous 512 KB DMA ----
            a_all = cpool.tile([128, NB * 128], F32)
            nc.scalar.dma_start(
                out=a_all[:].rearrange("q (b i h) -> q b i h", b=NB, i=NT),
                in_=Ad,
            )

            # scan ping-pong buffers; pad cols stay zero forever
            va = spool.tile([128, 192], F32, tag="va")
            vb = spool.tile([128, 192], F32, tag="vb")
            nc.vector.memset(va[:, 120:192], 0.0)
            nc.vector.memset(vb[:, 128:192], 0.0)

            NQ = 4        # X quarters per batch (finer DMA->matmul overlap)
            QT = NT // NQ  # K-tiles per X quarter

            def prep(b):
                """Emit loads + decay-weight chain + B weighting for batch b."""
                a1 = a_all[:, b * 128 : (b + 1) * 128]

                # ---- B load (contiguous 8 KB runs) ----
                bt = bpool.tile([128, NT * 128], F32)
                nc.scalar.dma_start(
                    out=bt[:].rearrange("q (i h n) -> q i h n", i=NT, h=NH),
                    in_=Bd[b].rearrange("(q i) h n -> q i h n", q=128),
                )

                # ---- X load (1 MB quarters; the final chunk of the last
                # batch is split in two so the tail matmul group after the
                # last DMA byte is half as long) ----
                # tiles are float32r so the BIR verifier accepts them as
                # fp32r-matmul operands; the DMA is a pure byte copy
                xs = []
                xr = Xd[b].rearrange("(q i) h p -> q i h p", q=128).bitcast(F32R)
                for quarter in range(NQ):
                    xt = xpool.tile([128, QT * 512], F32R, tag=f"x{quarter}")
                    if b == NB - 1 and quarter == NQ - 1:
                        ht = QT // 2
                        for eighth in range(2):
                            nc.sync.dma_start(
                                out=xt[:, eighth * ht * 512 : (eighth + 1) * ht * 512]
                                .rearrange("q (i h p) -> q i h p", i=ht, h=NH),
                                in_=xr[
                                    :,
                                    quarter * QT + eighth * ht : quarter * QT
                                    + (eighth + 1) * ht,
                                ],
                            )
                    else:
                        nc.sync.dma_start(
                            out=xt[:].rearrange("q (i h p) -> q i h p", i=QT, h=NH),
                            in_=xr[:, quarter * QT : (quarter + 1) * QT],
                        )
                    xs.append(xt)

                # ---- strict suffix over i (16 groups of 8 cols) ----
                nc.vector.tensor_copy(va[:, 0:120], a1[:, 8:128])
                nc.vector.tensor_add(vb[:, 0:128], va[:, 0:128], va[:, 8:136])
                nc.vector.tensor_add(va[:, 0:128], vb[:, 0:128], vb[:, 16:144])
                nc.vector.tensor_add(vb[:, 0:128], va[:, 0:128], va[:, 32:160])
                nc.vector.tensor_add(va[:, 0:128], vb[:, 0:128], vb[:, 64:192])

                # row totals T[q, h] = strict_suffix(i=0) + A(i=0)
                tt = wpool.tile([128, 8], F32, tag="tt")
                nc.vector.tensor_add(tt[:], va[:, 0:8], a1[:, 0:8])
                # carry[q, h] = sum_{q' > q} T[q', h]  (partition-dim suffix)
                pc = pcpool.tile([128, 8], F32, tag="pc")
                nc.tensor.matmul(pc[:], ltri[:], tt[:], start=True, stop=True)

                # w = exp(within-row suffix + carry)
                wpre = wpool.tile([128, 128], F32, tag="wpre")
                nc.vector.tensor_add(
                    wpre[:].rearrange("q (i h) -> q i h", i=NT),
                    va[:, 0:128].rearrange("q (i h) -> q i h", i=NT),
                    pc[:].unsqueeze(1).broadcast_to((128, NT, 8)),
                )
                w = wpool.tile([128, 128], F32, tag="w")
                nc.scalar.activation(w[:], wpre[:], mybir.ActivationFunctionType.Exp)

                # ---- decay weighting of B (broadcast over n) ----
                # output dtype float32r (DVE rounds) so the fp32r matmul
                # verifier accepts bw as a stationary operand
                bw = bwpool.tile([128, NT * 128], F32R)
                nc.vector.tensor_mul(
                    bw[:].rearrange("q (ih n) -> q ih n", n=DN),
                    bt[:].rearrange("q (ih n) -> q ih n", n=DN),
                    w[:].unsqueeze(2).broadcast_to((128, 128, DN)),
                )
                return bw, xs

            def mains(b, bw, xs):
                """16 accumulating matmuls: stationary Bw slice, moving X slice.

                float32r: single-pass fp32 matmul (4x faster than the 2x
                half-speed passes of plain fp32; full rate at N>=256)."""
                pm = pmpool.tile([128, 512], F32, tag="pm")
                for i in range(NT):
                    xt = xs[i // QT]
                    ii = i % QT
                    nc.tensor.matmul(
                        pm[:],
                        bw[:, i * 128 : (i + 1) * 128],
                        xt[:, ii * 512 : (ii + 1) * 512],
                        start=(i == 0),
                        stop=(i == NT - 1),
                    )
                return pm

            def outs(b, pm):
                """Band-transpose the 4 diagonal PSUM blocks into a common
                32-partition stage tile, then one contiguous HWDGE DMA."""
                stage = opool.tile([32, 512], F32, tag="stage")
                for j in range(4):
                    # 32x32 blockwise transpose of diagonal band j read
                    # straight from PSUM, relocated to partitions 0..32
                    nc.vector.transpose(
                        stage[:, 128 * j : 128 * (j + 1)],
                        pm[32 * j : 32 * j + 32, 128 * j : 128 * (j + 1)],
                    )
                nc.scalar.dma_start(out=Od[b], in_=stage[:])

            # software pipeline: keep DVE prep for b+1/b+2 ahead of b's outputs
            tiles = {b: prep(b) for b in range(2)}
            for b in range(NB):
                bw, xs = tiles.pop(b)
                pm = mains(b, bw, xs)
                if b + 2 < NB:
                    tiles[b + 2] = prep(b + 2)
                outs(b, pm)

    nc.compile()
    _NC_CACHE = nc
    return nc


def run(inputs, trace=False, tmpdir=None, trace_kwargs=None):
    """Run the SPMD kernel on 8 cores.  Returns (output, BassKernelResults)."""
    X = np.asarray(inputs["X"], dtype=np.float32)
    A = np.asarray(inputs["A"], dtype=np.float32)
    B = np.asarray(inputs["B"], dtype=np.float32)
    assert X.shape == (NCORES * NB, T, NH, DP), X.shape

    nc = _build()
    in_maps = []
    for c in range(NCORES):
        s = slice(c * NB, (c + 1) * NB)
        # comb A on the host: A_dev[q, b, i, h] = A[b, 16 q + i, h]
        a_comb = np.ascontiguousarray(
            A[s].reshape(NB, 128, T // 128, NH).transpose(1, 0, 2, 3)
        )
        in_maps.append(
            {
                "X": np.ascontiguousarray(X[s]),
                "A": a_comb,
                "B": np.ascontiguousarray(B[s]),
            }
        )
    kw = {}
    if trace:
        kw.update(trace=True, tmpdir=tmpdir, trace_kwargs=trace_kwargs or {})
    res = run_bass_kernel_spmd(nc, in_maps, core_ids=list(range(NCORES)), **kw)
    # O_dev[b, pp, 128j + 64e + 32k + 16e + n] = out[b, 2j+e, 32k+pp, n]
    raw = np.concatenate([res.results[c]["O"] for c in range(NCORES)], axis=0)
    r = raw.reshape(NCORES * NB, 32, 4, 2, 2, 32)  # (b, pp, j, e, k, m)
    out = np.empty((NCORES * NB, NH, DP, DN), dtype=np.float32)
    # head 2j (e=0) uses m 0:16; head 2j+1 (e=1) uses m 16:32
    # out[b, 2j+e, 32k+pp, n] = r[b, pp, j, e, k, 16e+n]
    he = r[:, :, :, 0, :, 0:16]  # (b, pp, j, k, n) even heads
    ho = r[:, :, :, 1, :, 16:32]  # odd heads
    out[:, 0::2] = he.transpose(0, 2, 3, 1, 4).reshape(NCORES * NB, 4, DP, DN)
    out[:, 1::2] = ho.transpose(0, 2, 3, 1, 4).reshape(NCORES * NB, 4, DP, DN)
    return out, res


def kernel(**inputs) -> np.ndarray:
    out, _ = run(inputs)
    return out



# revision 23
# speedup vs baseline: 1.1779x; 1.1779x over previous
"""Trainium2 Bass kernel for the chunked-scan final-state problem.

Math: the reference's chunked scan + inter-chunk segsum reduces exactly to
    out[b, h, p, n] = sum_t exp(sum_{t'>t} A[b, t', h]) * X[b, t, h, p] * B[b, t, h, n]
(input C is unused by the reference).  Per (b, h) this is a (64 x 2048) @
(2048 x 16) matmul with a decay weight folded into B.

Distribution: data-parallel over batch, 8 batches per core, 8 cores.

Layout trick ("comb" K-tiles): contraction tile i takes t in {16q + i},
q = partition.  Then every DMA is fully contiguous (partition q reads rows
16q..16q+15: X 8KB/quarter, B 8KB; A is host-combed so one 512KB DMA with
4KB runs loads every batch) and the decay suffix-sum becomes
  w[q, (i,h)] = exp( suffix_i(A_row q) + carry[q, h] )
where suffix_i is a 4-step shifted-add scan along the free dim and
carry = Lstrict^T @ row_totals is one small PE matmul over partitions.

Main matmuls (per batch, 16 K-tiles): stationary = weighted-B tile
(128 x 128 = all 8 heads), moving = X tile (128 x 512) -> PSUM (128 x 512)
accumulated over i; entry ((h'n), (h''p)).  Matmuls run in float32r
(single-pass fp32, 4x the plain-fp32 rate).  The diagonal h'=h'' blocks
are the per-head outputs in (n, p) orientation; four DVE 32x32 blockwise
band transposes gather them into one 32-partition stage tile, written to
DRAM with a single contiguous DMA per batch (host decodes the layout).
"""

import numpy as np

import concourse.bacc as bacc
import concourse.mybir as mybir
import concourse.tile as tile
from concourse.bass_utils import run_bass_kernel_spmd
from concourse.masks import make_lower_triangular

F32 = mybir.dt.float32
F32R = mybir.dt.float32r
NCORES = 8
NB = 8          # batches per core
T = 2048        # sequence length
NH = 8          # heads
DP = 64         # d_head
DN = 16         # d_state
NT = T // 128   # K-tiles of 128

_NC_CACHE = None


def _build():
    global _NC_CACHE
    if _NC_CACHE is not None:
        return _NC_CACHE

    nc = bacc.Bacc("TRN2", target_bir_lowering=False, debug=False)
    Xd = nc.dram_tensor("X", (NB, T, NH, DP), F32, kind="ExternalInput").ap()
    # A is pre-combed on the host: A_dev[q, b, i, h] = A[b, 16 q + i, h]
    # -> one DMA, 4 KB contiguous per partition
    Ad = nc.dram_tensor("A", (128, NB, NT, NH), F32, kind="ExternalInput").ap()
    Bd = nc.dram_tensor("B", (NB, T, NH, DN), F32, kind="ExternalInput").ap()
    # raw staged output: O[b, pp, c] = stage band dump; host decodes
    #   out[b, 2j+e, 32k+pp, n] = O[b, pp, 128j + 64e + 32k + 16e + n]
    Od = nc.dram_tensor("O", (NB, 32, 512), F32, kind="ExternalOutput").ap()

    with tile.TileContext(nc) as tc:
        with (
            tc.tile_pool(name="consts", bufs=1) as cpool,
            tc.tile_pool(name="scan", bufs=1) as spool,
            tc.tile_pool(name="wexp", bufs=3) as wpool,
            tc.tile_pool(name="bmat", bufs=3) as bpool,
            tc.tile_pool(name="bwp", bufs=3) as bwpool,
            tc.tile_pool(name="xmat", bufs=3) as xpool,
            tc.tile_pool(name="outs", bufs=3) as opool,
            tc.tile_pool(name="ps_carry", bufs=2, space="PSUM") as pcpool,
            tc.tile_pool(name="ps_main", bufs=4, space="PSUM") as pmpool,
        ):
            # strict lower-triangular constant: L[k, m] = 1 iff k > m
            ltri = cpool.tile([128, 128], F32)
            make_lower_triangular(nc, ltri[:], val=1.0, diag=False)

            # ---- A load: all batches in one contiguous 512 KB DMA ----
            a_all = cpool.tile([128, NB * 128], F32)
            nc.scalar.dma_start(
                out=a_all[:].rearrange("q (b i h) -> q b i h", b=NB, i=NT),
                in_=Ad,
            )

            # scan ping-pong buffers; pad cols stay zero forever
            va = spool.tile([128, 192], F32, tag="va")
            vb = spool.tile([128, 192], F32, tag="vb")
            nc.vector.memset(va[:, 120:192], 0.0)
            nc.vector.memset(vb[:, 128:192], 0.0)

            NQ = 4        # X quarters per batch (finer DMA->matmul overlap)
            QT = NT // NQ  # K-tiles per X quarter

            def prep(b):
                """Emit loads + decay-weight chain + B weighting for batch b."""
                a1 = a_all[:, b * 128 : (b + 1) * 128]

                # ---- B load (contiguous 8 KB runs) ----
                bt = bpool.tile([128, NT * 128], F32)
                nc.scalar.dma_start(
                    out=bt[:].rearrange("q (i h n) -> q i h n", i=NT, h=NH),
                    in_=Bd[b].rearrange("(q i) h n -> q i h n", q=128),
                )

                # ---- X load (1 MB quarters; the final chunk of the last
                # batch is split in two so the tail matmul group after the
                # last DMA byte is half as long) ----
                # tiles are float32r so the BIR verifier accepts them as
                # fp32r-matmul operands; the DMA is a pure byte copy
                xs = []
                xr = Xd[b].rearrange("(q i) h p -> q i h p", q=128).bitcast(F32R)
                for quarter in range(NQ):
                    xt = xpool.tile([128, QT * 512], F32R, tag=f"x{quarter}")
                    if b == NB - 1 and quarter == NQ - 1:
                        # single-K-tile chunks (256 KB) so the matmul chain
                        # after the very last DMA byte is one matmul long
                        for c in range(QT):
                            nc.sync.dma_start(
                                out=xt[:, c * 512 : (c + 1) * 512]
                                .rearrange("q (i h p) -> q i h p", i=1, h=NH),
                                in_=xr[:, quarter * QT + c : quarter * QT + c + 1],
                            )
                    else:
                        nc.sync.dma_start(
                            out=xt[:].rearrange("q (i h p) -> q i h p", i=QT, h=NH),
                            in_=xr[:, quarter * QT : (quarter + 1) * QT],
                        )
                    xs.append(xt)

                # ---- strict suffix over i (16 groups of 8 cols) ----
                nc.vector.tensor_copy(va[:, 0:120], a1[:, 8:128])
                nc.vector.tensor_add(vb[:, 0:128], va[:, 0:128], va[:, 8:136])
                nc.vector.tensor_add(va[:, 0:128], vb[:, 0:128], vb[:, 16:144])
                nc.vector.tensor_add(vb[:, 0:128], va[:, 0:128], va[:, 32:160])
                nc.vector.tensor_add(va[:, 0:128], vb[:, 0:128], vb[:, 64:192])

                # row totals T[q, h] = strict_suffix(i=0) + A(i=0)
                tt = wpool.tile([128, 8], F32, tag="tt")
                nc.vector.tensor_add(tt[:], va[:, 0:8], a1[:, 0:8])
                # carry[q, h] = sum_{q' > q} T[q', h]  (partition-dim suffix)
                pc = pcpool.tile([128, 8], F32, tag="pc")
                nc.tensor.matmul(pc[:], ltri[:], tt[:], start=True, stop=True)

                # w = exp(within-row suffix + carry)
                wpre = wpool.tile([128, 128], F32, tag="wpre")
                nc.vector.tensor_add(
                    wpre[:].rearrange("q (i h) -> q i h", i=NT),
                    va[:, 0:128].rearrange("q (i h) -> q i h", i=NT),
                    pc[:].unsqueeze(1).broadcast_to((128, NT, 8)),
                )
                w = wpool.tile([128, 128], F32, tag="w")
                nc.scalar.activation(w[:], wpre[:], mybir.ActivationFunctionType.Exp)

                # ---- decay weighting of B (broadcast over n) ----
                # output dtype float32r (DVE rounds) so the fp32r matmul
                # verifier accepts bw as a stationary operand
                bw = bwpool.tile([128, NT * 128], F32R)
                nc.vector.tensor_mul(
                    bw[:].rearrange("q (ih n) -> q ih n", n=DN),
                    bt[:].rearrange("q (ih n) -> q ih n", n=DN),
                    w[:].unsqueeze(2).broadcast_to((128, 128, DN)),
                )
                return bw, xs

            def mains(b, bw, xs):
                """16 accumulating matmuls: stationary Bw slice, moving X slice.

                float32r: single-pass fp32 matmul (4x faster than the 2x
                half-speed passes of plain fp32; full rate at N>=256)."""
                pm = pmpool.tile([128, 512], F32, tag="pm")
                for i in range(NT):
                    xt = xs[i // QT]
                    ii = i % QT
                    nc.tensor.matmul(
                        pm[:],
                        bw[:, i * 128 : (i + 1) * 128],
                        xt[:, ii * 512 : (ii + 1) * 512],
                        start=(i == 0),
                        stop=(i == NT - 1),
                    )
                return pm

            def outs(b, pm):
                """Band-transpose the 4 diagonal PSUM blocks into a common
                32-partition stage tile; each band's 16 KB DMA issues right
                after its transpose so transfers overlap the remaining
                transposes instead of waiting for all four."""
                stage = opool.tile([32, 512], F32, tag="stage")
                for j in range(4):
                    # 32x32 blockwise transpose of diagonal band j read
                    # straight from PSUM, relocated to partitions 0..32
                    nc.vector.transpose(
                        stage[:, 128 * j : 128 * (j + 1)],
                        pm[32 * j : 32 * j + 32, 128 * j : 128 * (j + 1)],
                    )
                    nc.scalar.dma_start(
                        out=Od[b][:, 128 * j : 128 * (j + 1)],
                        in_=stage[:, 128 * j : 128 * (j + 1)],
                    )

            # software pipeline: keep DVE prep for b+1/b+2 ahead of b's outputs
            tiles = {b: prep(b) for b in range(2)}
            for b in range(NB):
                bw, xs = tiles.pop(b)
                pm = mains(b, bw, xs)
                if b + 2 < NB:
                    tiles[b + 2] = prep(b + 2)
                outs(b, pm)

    nc.compile()
    _NC_CACHE = nc
    return nc


def run(inputs, trace=False, tmpdir=None, trace_kwargs=None):
    """Run the SPMD kernel on 8 cores.  Returns (output, BassKernelResults)."""
    X = np.asarray(inputs["X"], dtype=np.float32)
    A = np.asarray(inputs["A"], dtype=np.float32)
    B = np.asarray(inputs["B"], dtype=np.float32)
    assert X.shape == (NCORES * NB, T, NH, DP), X.shape

    nc = _build()
    in_maps = []
    for c in range(NCORES):
        s = slice(c * NB, (c + 1) * NB)
        # comb A on the host: A_dev[q, b, i, h] = A[b, 16 q + i, h]
        a_comb = np.ascontiguousarray(
            A[s].reshape(NB, 128, T // 128, NH).transpose(1, 0, 2, 3)
        )
        in_maps.append(
            {
                "X": np.ascontiguousarray(X[s]),
                "A": a_comb,
                "B": np.ascontiguousarray(B[s]),
            }
        )
    kw = {}
    if trace:
        kw.update(trace=True, tmpdir=tmpdir, trace_kwargs=trace_kwargs or {})
    res = run_bass_kernel_spmd(nc, in_maps, core_ids=list(range(NCORES)), **kw)
    # O_dev[b, pp, 128j + 64e + 32k + 16e + n] = out[b, 2j+e, 32k+pp, n]
    raw = np.concatenate([res.results[c]["O"] for c in range(NCORES)], axis=0)
    r = raw.reshape(NCORES * NB, 32, 4, 2, 2, 32)  # (b, pp, j, e, k, m)
    out = np.empty((NCORES * NB, NH, DP, DN), dtype=np.float32)
    # head 2j (e=0) uses m 0:16; head 2j+1 (e=1) uses m 16:32
    # out[b, 2j+e, 32k+pp, n] = r[b, pp, j, e, k, 16e+n]
    he = r[:, :, :, 0, :, 0:16]  # (b, pp, j, k, n) even heads
    ho = r[:, :, :, 1, :, 16:32]  # odd heads
    out[:, 0::2] = he.transpose(0, 2, 3, 1, 4).reshape(NCORES * NB, 4, DP, DN)
    out[:, 1::2] = ho.transpose(0, 2, 3, 1, 4).reshape(NCORES * NB, 4, DP, DN)
    return out, res


def kernel(**inputs) -> np.ndarray:
    out, _ = run(inputs)
    return out



# revision 24
# speedup vs baseline: 1.1955x; 1.0149x over previous
"""Trainium2 Bass kernel for the chunked-scan final-state problem.

Math: the reference's chunked scan + inter-chunk segsum reduces exactly to
    out[b, h, p, n] = sum_t exp(sum_{t'>t} A[b, t', h]) * X[b, t, h, p] * B[b, t, h, n]
(input C is unused by the reference).  Per (b, h) this is a (64 x 2048) @
(2048 x 16) matmul with a decay weight folded into B.

Distribution: data-parallel over batch, 8 batches per core, 8 cores.

Layout trick ("comb" K-tiles): contraction tile i takes t in {16q + i},
q = partition.  Then every DMA is fully contiguous (partition q reads rows
16q..16q+15: X 8KB/quarter, B 8KB; A is host-combed so one 512KB DMA with
4KB runs loads every batch) and the decay suffix-sum becomes
  w[q, (i,h)] = exp( suffix_i(A_row q) + carry[q, h] )
where suffix_i is a 4-step shifted-add scan along the free dim and
carry = Lstrict^T @ row_totals is one small PE matmul over partitions.

Main matmuls (per batch, 16 K-tiles): stationary = weighted-B tile
(128 x 128 = all 8 heads), moving = X tile (128 x 512) -> PSUM (128 x 512)
accumulated over i; entry ((h'n), (h''p)).  Matmuls run in float32r
(single-pass fp32, 4x the plain-fp32 rate).  The diagonal h'=h'' blocks
are the per-head outputs in (n, p) orientation; four DVE 32x32 blockwise
band transposes gather them into one 32-partition stage tile, written to
DRAM with a single contiguous DMA per batch (host decodes the layout).
"""

import numpy as np

import concourse.bacc as bacc
import concourse.mybir as mybir
import concourse.tile as tile
from concourse.bass_utils import run_bass_kernel_spmd
from concourse.masks import make_lower_triangular

F32 = mybir.dt.float32
F32R = mybir.dt.float32r
NCORES = 8
NB = 8          # batches per core
T = 2048        # sequence length
NH = 8          # heads
DP = 64         # d_head
DN = 16         # d_state
NT = T // 128   # K-tiles of 128

_NC_CACHE = None


def _build():
    global _NC_CACHE
    if _NC_CACHE is not None:
        return _NC_CACHE

    nc = bacc.Bacc("TRN2", target_bir_lowering=False, debug=False)
    Xd = nc.dram_tensor("X", (NB, T, NH, DP), F32, kind="ExternalInput").ap()
    # A is pre-combed on the host: A_dev[q, b, i, h] = A[b, 16 q + i, h]
    # -> one DMA, 4 KB contiguous per partition
    Ad = nc.dram_tensor("A", (128, NB, NT, NH), F32, kind="ExternalInput").ap()
    Bd = nc.dram_tensor("B", (NB, T, NH, DN), F32, kind="ExternalInput").ap()
    # raw staged output: O[b, pp, c] = stage band dump; host decodes
    #   out[b, 2j+e, 32k+pp, n] = O[b, pp, 128j + 64e + 32k + 16e + n]
    Od = nc.dram_tensor("O", (NB, 32, 512), F32, kind="ExternalOutput").ap()

    with tile.TileContext(nc) as tc:
        with (
            tc.tile_pool(name="consts", bufs=1) as cpool,
            tc.tile_pool(name="scan", bufs=1) as spool,
            tc.tile_pool(name="wexp", bufs=3) as wpool,
            tc.tile_pool(name="bmat", bufs=3) as bpool,
            tc.tile_pool(name="bwp", bufs=3) as bwpool,
            tc.tile_pool(name="xmat", bufs=3) as xpool,
            tc.tile_pool(name="outs", bufs=3) as opool,
            tc.tile_pool(name="ps_carry", bufs=2, space="PSUM") as pcpool,
            tc.tile_pool(name="ps_main", bufs=4, space="PSUM") as pmpool,
        ):
            # strict lower-triangular constant: L[k, m] = 1 iff k > m
            ltri = cpool.tile([128, 128], F32)
            make_lower_triangular(nc, ltri[:], val=1.0, diag=False)

            # ---- A load: all batches in one contiguous 512 KB DMA ----
            a_all = cpool.tile([128, NB * 128], F32)
            nc.scalar.dma_start(
                out=a_all[:].rearrange("q (b i h) -> q b i h", b=NB, i=NT),
                in_=Ad,
            )

            # scan ping-pong buffers; pad cols stay zero forever
            va = spool.tile([128, 192], F32, tag="va")
            vb = spool.tile([128, 192], F32, tag="vb")
            nc.vector.memset(va[:, 120:192], 0.0)
            nc.vector.memset(vb[:, 128:192], 0.0)

            NQ = 4        # X quarters per batch (finer DMA->matmul overlap)
            QT = NT // NQ  # K-tiles per X quarter

            def prep(b):
                """Emit loads + decay-weight chain + B weighting for batch b."""
                a1 = a_all[:, b * 128 : (b + 1) * 128]

                # ---- B load (contiguous 8 KB runs) ----
                bt = bpool.tile([128, NT * 128], F32)
                nc.scalar.dma_start(
                    out=bt[:].rearrange("q (i h n) -> q i h n", i=NT, h=NH),
                    in_=Bd[b].rearrange("(q i) h n -> q i h n", q=128),
                )

                # ---- X load (1 MB quarters; the final chunk of the last
                # batch is split in two so the tail matmul group after the
                # last DMA byte is half as long) ----
                # tiles are float32r so the BIR verifier accepts them as
                # fp32r-matmul operands; the DMA is a pure byte copy
                xs = []
                xr = Xd[b].rearrange("(q i) h p -> q i h p", q=128).bitcast(F32R)
                for quarter in range(NQ):
                    xt = xpool.tile([128, QT * 512], F32R, tag=f"x{quarter}")
                    if b == NB - 1 and quarter == NQ - 1:
                        # single-K-tile chunks (256 KB) so the matmul chain
                        # after the very last DMA byte is one matmul long
                        for c in range(QT):
                            nc.sync.dma_start(
                                out=xt[:, c * 512 : (c + 1) * 512]
                                .rearrange("q (i h p) -> q i h p", i=1, h=NH),
                                in_=xr[:, quarter * QT + c : quarter * QT + c + 1],
                            )
                    else:
                        nc.sync.dma_start(
                            out=xt[:].rearrange("q (i h p) -> q i h p", i=QT, h=NH),
                            in_=xr[:, quarter * QT : (quarter + 1) * QT],
                        )
                    xs.append(xt)

                # ---- strict suffix over i (16 groups of 8 cols) ----
                nc.vector.tensor_copy(va[:, 0:120], a1[:, 8:128])
                nc.vector.tensor_add(vb[:, 0:128], va[:, 0:128], va[:, 8:136])
                nc.vector.tensor_add(va[:, 0:128], vb[:, 0:128], vb[:, 16:144])
                nc.vector.tensor_add(vb[:, 0:128], va[:, 0:128], va[:, 32:160])
                nc.vector.tensor_add(va[:, 0:128], vb[:, 0:128], vb[:, 64:192])

                # row totals T[q, h] = strict_suffix(i=0) + A(i=0)
                tt = wpool.tile([128, 8], F32, tag="tt")
                nc.vector.tensor_add(tt[:], va[:, 0:8], a1[:, 0:8])
                # carry[q, h] = sum_{q' > q} T[q', h]  (partition-dim suffix)
                pc = pcpool.tile([128, 8], F32, tag="pc")
                nc.tensor.matmul(pc[:], ltri[:], tt[:], start=True, stop=True)

                # w = exp(within-row suffix + carry)
                wpre = wpool.tile([128, 128], F32, tag="wpre")
                nc.vector.tensor_add(
                    wpre[:].rearrange("q (i h) -> q i h", i=NT),
                    va[:, 0:128].rearrange("q (i h) -> q i h", i=NT),
                    pc[:].unsqueeze(1).broadcast_to((128, NT, 8)),
                )
                w = wpool.tile([128, 128], F32, tag="w")
                nc.scalar.activation(w[:], wpre[:], mybir.ActivationFunctionType.Exp)

                # ---- decay weighting of B (broadcast over n) ----
                # output dtype float32r (DVE rounds) so the fp32r matmul
                # verifier accepts bw as a stationary operand
                bw = bwpool.tile([128, NT * 128], F32R)
                nc.vector.tensor_mul(
                    bw[:].rearrange("q (ih n) -> q ih n", n=DN),
                    bt[:].rearrange("q (ih n) -> q ih n", n=DN),
                    w[:].unsqueeze(2).broadcast_to((128, 128, DN)),
                )
                return bw, xs

            def mains(b, bw, xs):
                """16 accumulating matmuls: stationary Bw slice, moving X slice.

                float32r: single-pass fp32 matmul (4x faster than the 2x
                half-speed passes of plain fp32; full rate at N>=256)."""
                pm = pmpool.tile([128, 512], F32, tag="pm")
                for i in range(NT):
                    xt = xs[i // QT]
                    ii = i % QT
                    nc.tensor.matmul(
                        pm[:],
                        bw[:, i * 128 : (i + 1) * 128],
                        xt[:, ii * 512 : (ii + 1) * 512],
                        start=(i == 0),
                        stop=(i == NT - 1),
                    )
                return pm

            def outs(b, pm):
                """Band-transpose the 4 diagonal PSUM blocks into a common
                32-partition stage tile, then one contiguous HWDGE DMA.
                (A per-band DMA split was measured slower: 4 x 549 ns issue
                slots on the ACT engine outweigh overlapping 16 KB bursts.)"""
                stage = opool.tile([32, 512], F32, tag="stage")
                for j in range(4):
                    # 32x32 blockwise transpose of diagonal band j read
                    # straight from PSUM, relocated to partitions 0..32
                    nc.vector.transpose(
                        stage[:, 128 * j : 128 * (j + 1)],
                        pm[32 * j : 32 * j + 32, 128 * j : 128 * (j + 1)],
                    )
                nc.scalar.dma_start(out=Od[b], in_=stage[:])

            # software pipeline: keep DVE prep for b+1/b+2 ahead of b's outputs
            tiles = {b: prep(b) for b in range(2)}
            for b in range(NB):
                bw, xs = tiles.pop(b)
                pm = mains(b, bw, xs)
                if b + 2 < NB:
                    tiles[b + 2] = prep(b + 2)
                outs(b, pm)

    nc.compile()
    _NC_CACHE = nc
    return nc


def run(inputs, trace=False, tmpdir=None, trace_kwargs=None):
    """Run the SPMD kernel on 8 cores.  Returns (output, BassKernelResults)."""
    X = np.asarray(inputs["X"], dtype=np.float32)
    A = np.asarray(inputs["A"], dtype=np.float32)
    B = np.asarray(inputs["B"], dtype=np.float32)
    assert X.shape == (NCORES * NB, T, NH, DP), X.shape

    nc = _build()
    in_maps = []
    for c in range(NCORES):
        s = slice(c * NB, (c + 1) * NB)
        # comb A on the host: A_dev[q, b, i, h] = A[b, 16 q + i, h]
        a_comb = np.ascontiguousarray(
            A[s].reshape(NB, 128, T // 128, NH).transpose(1, 0, 2, 3)
        )
        in_maps.append(
            {
                "X": np.ascontiguousarray(X[s]),
                "A": a_comb,
                "B": np.ascontiguousarray(B[s]),
            }
        )
    kw = {}
    if trace:
        kw.update(trace=True, tmpdir=tmpdir, trace_kwargs=trace_kwargs or {})
    res = run_bass_kernel_spmd(nc, in_maps, core_ids=list(range(NCORES)), **kw)
    # O_dev[b, pp, 128j + 64e + 32k + 16e + n] = out[b, 2j+e, 32k+pp, n]
    raw = np.concatenate([res.results[c]["O"] for c in range(NCORES)], axis=0)
    r = raw.reshape(NCORES * NB, 32, 4, 2, 2, 32)  # (b, pp, j, e, k, m)
    out = np.empty((NCORES * NB, NH, DP, DN), dtype=np.float32)
    # head 2j (e=0) uses m 0:16; head 2j+1 (e=1) uses m 16:32
    # out[b, 2j+e, 32k+pp, n] = r[b, pp, j, e, k, 16e+n]
    he = r[:, :, :, 0, :, 0:16]  # (b, pp, j, k, n) even heads
    ho = r[:, :, :, 1, :, 16:32]  # odd heads
    out[:, 0::2] = he.transpose(0, 2, 3, 1, 4).reshape(NCORES * NB, 4, DP, DN)
    out[:, 1::2] = ho.transpose(0, 2, 3, 1, 4).reshape(NCORES * NB, 4, DP, DN)
    return out, res


def kernel(**inputs) -> np.ndarray:
    out, _ = run(inputs)
    return out

